# revision 27
# baseline (speedup 1.0000x reference)
"""Multi-head self-attention (B=4, S=2048, D=1024, H=8) on 8 TRN2 NeuronCores.

Sharding: core c -> batch b=c//2, head-group g=c%2 (4 heads/core).
Each core computes its 4 heads' attention output [512, 2048] (transposed,
head-major); the host gathers/reassembles the full [B, S, D] output.

Notes on the math: the reference adds the source mask per-QUERY (constant
along the key axis) before a softmax over keys, so the mask cancels exactly;
encoder_output_embedding and the target mask are unused by the reference.
The kernel therefore computes pure softmax(q k^T / sqrt(dh)) v.
"""

import math
from contextlib import ExitStack

import numpy as np

import concourse.bacc as bacc
import concourse.tile as tile
from concourse import mybir
from concourse.bass_utils import run_bass_kernel_spmd

N_CORES = 8
B, S, D, H = 4, 2048, 1024, 8
DH = 128                    # head dim
HPC = 4                     # heads per core
DHG = HPC * DH              # 512: projected width per core
SCALE = 1.0 / math.sqrt(DH)

F32 = mybir.dt.float32
F16 = mybir.dt.float16

TRACE = False               # test.py flips this for profiling runs
_CACHE = {}


def _emit(tc, nc, xt_ap, wq_ap, wk_ap, wv_ap, out_ap):
    KT = S // 128            # 16 key tiles
    ND = D // 128            # 8 contraction tiles

    with ExitStack() as ctx:
        p_xt = ctx.enter_context(tc.tile_pool(name="xt", bufs=32))
        p_w = ctx.enter_context(tc.tile_pool(name="w", bufs=ND))
        p_qt = ctx.enter_context(tc.tile_pool(name="qt", bufs=2))
        p_v = ctx.enter_context(tc.tile_pool(name="v", bufs=KT))
        p_exp = ctx.enter_context(tc.tile_pool(name="exp", bufs=5))
        p_out = ctx.enter_context(tc.tile_pool(name="o", bufs=2))
        p_rc = ctx.enter_context(tc.tile_pool(name="rc", bufs=2))
        p_const = ctx.enter_context(tc.tile_pool(name="const", bufs=1))
        ps_mm = ctx.enter_context(tc.tile_pool(name="psmm", bufs=2, space="PSUM"))
        ps_pv = ctx.enter_context(tc.tile_pool(name="pspv", bufs=1, space="PSUM"))
        ps_pj = ctx.enter_context(tc.tile_pool(name="pspj", bufs=2, space="PSUM"))
        p_dram = ctx.enter_context(tc.tile_pool(name="dram", bufs=2, space="DRAM"))

        ones = p_const.tile([128, 1], F16, tag="ones")
        nc.vector.memset(ones[:], 1.0)

        # DMAs in consumer-priority order, 128KB chunks spread across the 16
        # HW queues (~22GB/s each): wv + first xt column-block feed the V
        # stage within ~13us; wq/wk arrive before their projections need them.
        NSB = S // 512
        xts = [[None] * NSB for _ in range(ND)]
        ws = {"wv": [None] * ND, "wq": [None] * ND, "wk": [None] * ND}

        def dma_w(name, ap, d):
            t = p_w.tile([128, DHG], F16, tag=name)
            nc.sync.dma_start(t[:], ap[d * 128:(d + 1) * 128, :])
            ws[name][d] = t

        def dma_xt(d, sb):
            t = p_xt.tile([128, 512], F16, tag="xt")
            nc.sync.dma_start(
                t[:], xt_ap[d * 128:(d + 1) * 128, sb * 512:(sb + 1) * 512]
            )
            xts[d][sb] = t

        for d in range(ND):
            dma_w("wv", wv_ap, d)
        for d in range(ND):
            dma_xt(d, 0)
        for d in range(ND):
            dma_xt(d, 1)
        for d in range(ND):
            dma_w("wq", wq_ap, d)
        for d in range(ND):
            dma_xt(d, 2)
        for d in range(ND):
            dma_xt(d, 3)
        for d in range(ND):
            dma_w("wk", wk_ap, d)

        def proj_steps(h):
            """Yield once per PE-chunk of head h's q/k projections."""
            qt = p_qt.tile([128, S], F16, tag="qt")
            kt = p_qt.tile([128, S], F16, tag="kt")
            gi = 0
            for dst, wname, scale in ((qt, "wq", SCALE), (kt, "wk", None)):
                for sb in range(S // 512):
                    # head 0 projects serially (before any attention): borrow
                    # the idle sT slots for deeper group pipelining there
                    if h == 0 and gi % 2 == 0:
                        ps = ps_mm.tile([128, 512], F32, tag="sT")
                    else:
                        ps = ps_pj.tile([128, 512], F32, tag="proj")
                    gi += 1
                    for d in range(ND):
                        nc.tensor.matmul(
                            ps[:],
                            ws[wname][d][:, h * 128:(h + 1) * 128],
                            xts[d][sb][:],
                            start=(d == 0),
                            stop=(d == ND - 1),
                        )
                        if d % 2 == 1:
                            yield None
                    dsl = dst[:, sb * 512:(sb + 1) * 512]
                    if scale is not None:
                        nc.vector.tensor_scalar_mul(dsl, ps[:], scale)
                    else:
                        nc.vector.tensor_copy(dsl, ps[:])
            while True:
                yield (qt, kt)

        def attention_head(h, qt, kt, next_proj):
            """Phase B for head h; drip-feeds next head's projection matmuls
            into the ACT-paced kt loop."""
            for qb in range(S // 1024):
                pv = ps_pv.tile([128, 1024], F32, tag="pv")
                q0 = qb * 1024
                ets = {}
                acc = [None]

                def qk_step(k):
                    st_ps = ps_mm.tile([128, 1024], F32, tag="sT")
                    for hf in range(2):
                        nc.tensor.matmul(
                            st_ps[:, hf * 512:(hf + 1) * 512],
                            kt[:, k * 128:(k + 1) * 128],
                            qt[:, q0 + hf * 512:q0 + (hf + 1) * 512],
                            start=True,
                            stop=True,
                        )
                    et = p_exp.tile([128, 1024], F16, tag="exp")
                    nc.scalar.activation(
                        et[:], st_ps[:], mybir.ActivationFunctionType.Exp
                    )
                    ets[k] = et

                def pv_step(k):
                    et = ets.pop(k)
                    for hf in range(2):
                        sl = slice(hf * 512, (hf + 1) * 512)
                        nc.tensor.matmul(
                            pv[:, sl],
                            vts[k][:, h * 128:(h + 1) * 128],
                            et[:, sl],
                            start=(k == 0),
                            stop=(k == KT - 1),
                        )
                    if acc[0] is None:
                        acc[0] = et
                    else:
                        nc.vector.tensor_add(acc[0][:], acc[0][:], et[:])

                qk_step(0)
                for k in range(1, KT):
                    qk_step(k)
                    pv_step(k - 1)
                    if next_proj is not None:
                        next(next_proj)
                pv_step(KT - 1)

                # cross-partition reduce of the folded exp accumulator
                sms = []
                for hf in range(2):
                    sl = slice(hf * 512, (hf + 1) * 512)
                    sm = ps_pj.tile([1, 512], F32, tag="proj")
                    nc.tensor.matmul(sm[:], ones[:], acc[0][:, sl], start=True, stop=True)
                    sms.append(sm)

                # normalization: sums -> DRAM -> [128,8] so the reciprocal
                # runs wide (a [1,1024] DVE op is ~6.5us), then broadcast
                # back through DRAM; PSUM tiles are freed early.
                sm_sb = p_rc.tile([1, 1024], F32, tag="sm_sb")
                for hf in range(2):
                    nc.vector.tensor_copy(sm_sb[:, hf * 512:(hf + 1) * 512], sms[hf][:])
                sm2 = p_rc.tile([128, 8], F32, tag="sm2")
                nc.sync.dma_start(sm2[:], sm_sb[:])
                rc2 = p_rc.tile([128, 8], F32, tag="rc2")
                nc.vector.reciprocal(rc2[:], sm2[:])
                r2dram = p_dram.tile([1, 1024], F32, tag="r2dram")
                nc.sync.dma_start(r2dram[:].rearrange("a (p c) -> (a p) c", p=128), rc2[:])
                rbc = p_rc.tile([128, 1024], F32, tag="rbc")
                nc.sync.dma_start(rbc[:], r2dram[0:1, :].to_broadcast((128, 1024)))

                ob = p_out.tile([128, 1024], F32, tag="o")
                if h == HPC - 1 and qb == 1:
                    nc.vector.tensor_mul(ob[:], pv[:], rbc[:])
                else:
                    nc.vector.tensor_copy(ob[:], pv[:])
                    nc.vector.tensor_mul(ob[:], ob[:], rbc[:])
                for hf in range(2):
                    nc.sync.dma_start(
                        out_ap[h * 128:(h + 1) * 128,
                               qb * 1024 + hf * 512:qb * 1024 + (hf + 1) * 512],
                        ob[:, hf * 512:(hf + 1) * 512],
                    )

        # head 0's projections run serially (nothing to hide them under);
        # heads 1..3 project inside the previous head's attention loop.
        # V = x @ wv, natural [s, hd] layout (f16 for the PV matmul)
        vts = []
        for st in range(KT):
            ps = ps_mm.tile([128, DHG], F32, tag="sT")
            for d in range(ND):
                nc.tensor.matmul(
                    ps[:],
                    xts[d][st // 4][:, (st % 4) * 128:(st % 4 + 1) * 128],
                    ws["wv"][d][:],
                    start=(d == 0),
                    stop=(d == ND - 1),
                )
            vt = p_v.tile([128, DHG], F16, tag="v")
            nc.vector.tensor_copy(vt[:], ps[:])
            vts.append(vt)

        gen = proj_steps(0)
        res = None
        while not isinstance(res, tuple):
            res = next(gen)
        qt, kt = res
        for h in range(HPC):
            nxt = proj_steps(h + 1) if h + 1 < HPC else None
            attention_head(h, qt, kt, nxt)
            if nxt is not None:
                res = None
                while not isinstance(res, tuple):
                    res = next(nxt)
                qt, kt = res

def _build():
    nc = bacc.Bacc(
        "TRN2",
        target_bir_lowering=False,
        debug=False,
        enable_asserts=False,
        num_devices=N_CORES,
    )
    xt_ap = nc.dram_tensor("xt", [D, S], F16, kind="ExternalInput").ap()
    wq_ap = nc.dram_tensor("wq", [D, DHG], F16, kind="ExternalInput").ap()
    wk_ap = nc.dram_tensor("wk", [D, DHG], F16, kind="ExternalInput").ap()
    wv_ap = nc.dram_tensor("wv", [D, DHG], F16, kind="ExternalInput").ap()
    out_ap = nc.dram_tensor("out", [DHG, S], F32, kind="ExternalOutput").ap()
    with tile.TileContext(nc) as tc:
        _emit(tc, nc, xt_ap, wq_ap, wk_ap, wv_ap, out_ap)
    nc.compile()
    return nc


def _shard_inputs(inputs):
    x = np.ascontiguousarray(np.asarray(inputs["input_embeddings"], dtype=np.float32))
    wq = np.asarray(inputs["w_query"], dtype=np.float32)
    wk = np.asarray(inputs["w_key"], dtype=np.float32)
    wv = np.asarray(inputs["w_value"], dtype=np.float32)

    def gather(w, g):
        # head h occupies the strided cols d = hd*8 + h; regroup head-major
        w4 = w.reshape(D, DH, H)[:, :, g * HPC:(g + 1) * HPC]   # (D, hd, hl)
        return np.ascontiguousarray(w4.transpose(0, 2, 1).reshape(D, DHG).astype(np.float16))

    in_maps = []
    for c in range(N_CORES):
        b, g = divmod(c, 2)
        in_maps.append(
            {
                "xt": np.ascontiguousarray(x[b].T.astype(np.float16)),
                "wq": gather(wq, g),
                "wk": gather(wk, g),
                "wv": gather(wv, g),
            }
        )
    return in_maps


def kernel(**inputs):
    nc = _CACHE.get("nc")
    if nc is None:
        nc = _CACHE["nc"] = _build()
    in_maps = _shard_inputs(inputs)
    res = run_bass_kernel_spmd(
        nc, in_maps, core_ids=list(range(N_CORES)), trace=TRACE
    )
    _CACHE["last_result"] = res
    out = np.empty((B, S, DH, H), dtype=np.float32)
    for c in range(N_CORES):
        b, g = divmod(c, 2)
        o = res.results[c]["out"].reshape(HPC, DH, S)            # (hl, hd, s)
        out[b, :, :, g * HPC:(g + 1) * HPC] = o.transpose(2, 1, 0)
    return out.reshape(B, S, D)
